# revision 14
# baseline (speedup 1.0000x reference)
"""MoE transformer block (router top-2 + 8 experts + shared SwiGLU expert) on 8 trn2 cores.

Sharding: token-parallel. Each core gets 512 of the 4096 tokens and computes the
full mixture for its tokens (dense masked-combine over all 8 experts, identical
math to the reference). Weights are replicated; no collectives are needed.

Device kernel layout (per core, P=128 partitions):
  xT   [C, NT]  : tokens on the free dim, d_model on partitions (8 chunks of 128)
  gu_e = gate_up_w[e].T-free matmuls -> [2H partitions-chunks, NT free]
  h_e  = silu(gate) * up * combine_w  (combine broadcast via a K=1 ones matmul)
  y   += h_e.T-chunks @ down_w[e]     -> [NT partitions-chunks, C free]
  shared expert identically with host-pre-transposed weights.
All big matmuls use float32r (full-rate fp32 on the PE when free dim >= 256).
"""

import contextlib
import ctypes
import os
import sys
import types

sys.path.insert(0, "/opt/trn_rl_repo")

import numpy as np


def _install_ntff_shim():
    """Provide antenv.axon_hooks (missing in this image) so that
    run_bass_kernel_spmd(trace=True) can drive NTFF profiling through
    libaxon_pjrt.so's C ABI. Degrades to hook=None when the .so or its
    symbols are absent (bass_utils then skips tracing gracefully)."""
    if "antenv.axon_hooks" in sys.modules:
        return
    hook = None
    so_path = "/opt/axon/libaxon_pjrt.so"
    try:
        if os.path.exists(so_path):
            lib = ctypes.CDLL(so_path)
            if hasattr(lib, "axon_start_nrt_profile"):
                lib.axon_start_nrt_profile.argtypes = [
                    ctypes.POINTER(ctypes.c_int64),
                    ctypes.c_size_t,
                ]
                lib.axon_start_nrt_profile.restype = ctypes.c_int64
                lib.axon_stop_nrt_profile.argtypes = [ctypes.c_char_p]
                lib.axon_stop_nrt_profile.restype = ctypes.c_int64

                @contextlib.contextmanager
                def _hook(output_dir, device_ids):
                    import jax

                    jax.devices()
                    if device_ids:
                        ids = (ctypes.c_int64 * len(device_ids))(*device_ids)
                        rc = lib.axon_start_nrt_profile(ids, len(device_ids))
                    else:
                        rc = lib.axon_start_nrt_profile(None, 0)
                    if rc != 0:
                        raise RuntimeError(f"axon_start_nrt_profile rc={rc}")
                    try:
                        yield
                    finally:
                        n = lib.axon_stop_nrt_profile(str(output_dir).encode())
                        print(f"ntff profile: {n} file(s) -> {output_dir}", file=sys.stderr)

                hook = _hook
    except OSError:
        hook = None

    mod = types.ModuleType("antenv.axon_hooks")
    mod._hook = hook
    mod.get_axon_ntff_profile_hook = lambda: mod._hook

    def _set(h):
        mod._hook = h

    mod.set_axon_ntff_profile_hook = _set
    sys.modules["antenv.axon_hooks"] = mod


_install_ntff_shim()

import concourse.bass as bass
import concourse.mybir as mybir
import concourse.tile as tile
from concourse import bacc
from concourse.bass_utils import run_bass_kernel_spmd
from concourse.masks import make_identity

P = 128
F32 = mybir.dt.float32
F32R = mybir.dt.float32r
AF = mybir.ActivationFunctionType
ALU = mybir.AluOpType

# full problem dims
B, T, C_FULL = 4, 1024, 1024
E_FULL, H_FULL, HS_FULL = 8, 512, 2048
N_CORES = 8
N_TOK = B * T


def emit_moe(nc, tc, dims, aps):
    """Emit the per-core MoE kernel. dims: NT, C, E, H, HS. aps: dict of DRAM APs."""
    NT, C, E, H, HS = dims["NT"], dims["C"], dims["E"], dims["H"], dims["HS"]
    KC = C // P          # contraction chunks over d_model
    NCH = NT // P        # token chunks (tokens on partitions)
    JCH = 2 * H // P     # gate_up output chunks (0..JCH/2-1 gate, rest up)
    GCH = JCH // 2
    HCH = H // P         # expert hidden chunks
    HSCH = HS // P       # shared hidden chunks
    CW = min(512, C)     # matmul moving width for C-sized free dims
    CCH = C // CW
    NW = min(512, NT)    # moving width for token free dim
    assert NW == NT, "single token-span per core assumed"

    xT_d, rwT_d, guw_d, dw_d, sgwT_d, suwT_d, sdwT_d, y_d = (
        aps["xT"], aps["rwT"], aps["gate_up_w"], aps["down_w"],
        aps["sgwT"], aps["suwT"], aps["sdwT"], aps["y"],
    )
    xT32_d = aps["xT32"]

    # ---- pools ----
    import contextlib
    ctx = contextlib.ExitStack()

    # persistent tiles: one slot per tag in a bufs=1 pool
    res = ctx.enter_context(tc.tile_pool(name="res", bufs=1))
    xT_sb = res.tile([P, KC, NT], F32R, name="xt", tag="xt")
    y_sb = res.tile([P, NCH, C], F32, name="ysb", tag="ysb")
    comb_rows = res.tile([1, E, NT], F32, name="combt", tag="combt")
    rw_sb = res.tile([P, KC, E], F32, name="rwsb", tag="rwsb")
    ident = res.tile([P, P], F32, name="ident", tag="ident")

    make_identity(nc, ident)

    w1024 = ctx.enter_context(tc.tile_pool(name="w1024", bufs=9))    # guw + shared gate/up weight tiles
    dwp = ctx.enter_context(tc.tile_pool(name="dwp", bufs=5))        # down_w tiles
    sdwp = ctx.enter_context(tc.tile_pool(name="sdwp", bufs=4))      # shared down tiles
    wbp = ctx.enter_context(tc.tile_pool(name="wbp", bufs=2))        # combine broadcast tiles
    sgp = ctx.enter_context(tc.tile_pool(name="sgp", bufs=4))        # sigmoid tiles
    upp = ctx.enter_context(tc.tile_pool(name="upp", bufs=8))        # expert up/h tiles
    ssgp = ctx.enter_context(tc.tile_pool(name="ssgp", bufs=HSCH + 2))  # shared act tiles
    rsm = ctx.enter_context(tc.tile_pool(name="rsm", bufs=2))        # router small tiles
    rxp = ctx.enter_context(tc.tile_pool(name="rxp", bufs=2))        # fp32 x slices for router
    pgu = ctx.enter_context(tc.tile_pool(name="pgu", bufs=3, space="PSUM"))
    pdn = ctx.enter_context(tc.tile_pool(name="pdn", bufs=4, space="PSUM"))
    prt = ctx.enter_context(tc.tile_pool(name="prt", bufs=1, space="PSUM"))

    # ---- input DMAs: xT and router weights ----
    for k in range(KC):
        nc.sync.dma_start(out=xT_sb[:, k, :], in_=xT_d[k * P:(k + 1) * P, :])
        nc.sync.dma_start(out=rw_sb[:, k, :], in_=rwT_d[k * P:(k + 1) * P, :])

    # ---- router: logits -> top-2 mask -> sigmoid gates -> combT [E, NT] ----
    for n in range(NCH):
        lg_ps = prt.tile([P, E], F32, name="lg", tag="prt")
        # exact fp32 logits: top-2 selection must not flip on fp32r noise
        rx = rxp.tile([P, KC, P], F32, name="rx", tag="rx")
        for k in range(KC):
            nc.sync.dma_start(out=rx[:, k, :], in_=xT32_d[k * P:(k + 1) * P, n * P:(n + 1) * P])
        for k in range(KC):
            nc.tensor.matmul(
                lg_ps,
                lhsT=rx[:, k, :],
                rhs=rw_sb[:, k, :],
                start=(k == 0), stop=(k == KC - 1),
            )
        logits = rsm.tile([P, E], F32, name="logits", tag="lgs")
        nc.vector.tensor_copy(logits, lg_ps)
        m1 = rsm.tile([P, 1], F32, name="m1", tag="m1")
        nc.vector.reduce_max(m1, logits, axis=mybir.AxisListType.X)
        # (logits == m1) * -1e30  -> knock out the argmax
        eqb = rsm.tile([P, E], F32, name="eqb", tag="eqb")
        nc.vector.tensor_scalar(eqb, logits, m1, -1.0e30, op0=ALU.is_equal, op1=ALU.mult)
        masked = rsm.tile([P, E], F32, name="masked", tag="msk")
        nc.vector.tensor_add(masked, logits, eqb)
        m2 = rsm.tile([P, 1], F32, name="m2", tag="m2")
        nc.vector.reduce_max(m2, masked, axis=mybir.AxisListType.X)
        mask2 = rsm.tile([P, E], F32, name="mask2", tag="msk2")
        nc.vector.tensor_scalar(mask2, logits, m2, None, op0=ALU.is_ge)
        sig = rsm.tile([P, E], F32, name="sig", tag="sig")
        nc.scalar.activation(sig, logits, AF.Sigmoid)
        comb = rsm.tile([P, E], F32, name="comb", tag="comb")
        nc.vector.tensor_mul(comb, sig, mask2)
        # transpose each expert column [P, 1] -> a [1, P] row at partition 0
        for e in range(E):
            tp_ps = prt.tile([1, P], F32, name="tp", tag="prt")
            nc.tensor.transpose(tp_ps, comb[:, e:e + 1], ident)
            nc.vector.tensor_copy(comb_rows[0:1, e, n * P:(n + 1) * P], tp_ps)

    # ---- experts ----
    for e in range(E):
        # broadcast combine row e -> [P, NT]
        wb = wbp.tile([P, NT], F32, name="wb", tag="wb")
        nc.gpsimd.partition_broadcast(wb, comb_rows[0:1, e, :])

        guw_t = []
        for k in range(KC):
            g = w1024.tile([P, 2 * H], F32R, name=f"guw{k}", tag="w1024")
            nc.sync.dma_start(out=g, in_=guw_d[e, k * P:(k + 1) * P, :])
            guw_t.append(g)

        # process up chunks first, then gate chunks:
        #   h = silu(g) * (u*w) = (g * (u*w)) * sigmoid(g)
        up_t = []
        for j in list(range(GCH, JCH)) + list(range(GCH)):
            gu_ps = pgu.tile([P, NT], F32, name="gups", tag="pgu")
            for k in range(KC):
                nc.tensor.matmul(
                    gu_ps,
                    lhsT=guw_t[k][:, j * P:(j + 1) * P],
                    rhs=xT_sb[:, k, :],
                    start=(k == 0), stop=(k == KC - 1),
                )
            if j >= GCH:
                up = upp.tile([P, NT], F32R, name="up", tag="up")
                nc.vector.tensor_mul(up, gu_ps, wb)  # up * combine_w
                up_t.append(up)
            else:
                ut = up_t[j]
                nc.vector.tensor_mul(ut, gu_ps, ut)          # g * (u*w)
                sg = sgp.tile([P, NT], F32, name="sg", tag="sg")
                nc.scalar.activation(sg, gu_ps, AF.Sigmoid)  # sigmoid(g)
                nc.vector.tensor_mul(ut, sg, ut)             # h

        dw_t = []
        for k in range(HCH):
            d = dwp.tile([P, C], F32R, name=f"dw{k}", tag="dw")
            nc.sync.dma_start(out=d, in_=dw_d[e, k * P:(k + 1) * P, :])
            dw_t.append(d)
        for ch in range(CCH):
            for n in range(NCH):
                d_ps = pdn.tile([P, CW], F32, name="dps", tag="pdn")
                for k in range(HCH):
                    nc.tensor.matmul(
                        d_ps,
                        lhsT=up_t[k][:, n * P:(n + 1) * P],
                        rhs=dw_t[k][:, ch * CW:(ch + 1) * CW],
                        start=(k == 0), stop=(k == HCH - 1),
                    )
                ysl = y_sb[:, n, ch * CW:(ch + 1) * CW]
                if e == 0:
                    nc.vector.tensor_copy(ysl, d_ps)
                else:
                    nc.vector.tensor_add(ysl, ysl, d_ps)

    # ---- shared expert ----
    HS_BLK = min(8, HSCH)  # hs-chunk group per weight-tile residency
    ssg_t = [None] * HSCH
    for s, w_d in ((0, suwT_d), (1, sgwT_d)):  # up pass first, then gate pass
        for blk in range(0, HSCH, HS_BLK):
            sw_t = []
            for k in range(KC):
                w = w1024.tile([P, HS_BLK * P], F32R, name=f"sw{k}", tag="w1024")
                nc.sync.dma_start(
                    out=w, in_=w_d[k * P:(k + 1) * P, blk * P:(blk + HS_BLK) * P]
                )
                sw_t.append(w)
            for hsl in range(HS_BLK):
                hs = blk + hsl
                s_ps = pgu.tile([P, NT], F32, name="sps", tag="pgu")
                for k in range(KC):
                    nc.tensor.matmul(
                        s_ps,
                        lhsT=sw_t[k][:, hsl * P:(hsl + 1) * P],
                        rhs=xT_sb[:, k, :],
                        start=(k == 0), stop=(k == KC - 1),
                    )
                if s == 0:
                    ssg = ssgp.tile([P, NT], F32R, name="ssg", tag="ssg")
                    nc.vector.tensor_copy(ssg, s_ps)             # u
                    ssg_t[hs] = ssg
                else:
                    ut = ssg_t[hs]
                    nc.vector.tensor_mul(ut, s_ps, ut)           # g * u
                    ssig = sgp.tile([P, NT], F32, name="ssig", tag="sg")
                    nc.scalar.activation(ssig, s_ps, AF.Sigmoid)
                    nc.vector.tensor_mul(ut, ssig, ut)           # act

    for ch in range(CCH):
        d_ps = [pdn.tile([P, CW], F32, name=f"sdps{n}", tag="pdn") for n in range(NCH)]
        for k in range(HSCH):
            w = sdwp.tile([P, CW], F32R, name="sdw", tag="sdw")
            nc.sync.dma_start(
                out=w, in_=sdwT_d[k * P:(k + 1) * P, ch * CW:(ch + 1) * CW]
            )
            for n in range(NCH):
                nc.tensor.matmul(
                    d_ps[n],
                    lhsT=ssg_t[k][:, n * P:(n + 1) * P],
                    rhs=w,
                    start=(k == 0), stop=(k == HSCH - 1),
                )
        for n in range(NCH):
            ysl = y_sb[:, n, ch * CW:(ch + 1) * CW]
            nc.vector.tensor_add(ysl, ysl, d_ps[n])

    # ---- store ----
    for n in range(NCH):
        nc.sync.dma_start(out=y_d[n * P:(n + 1) * P, :], in_=y_sb[:, n, :])

    ctx.close()


def build(dims, debug=False, enable_asserts=False):
    NT, C, E, H, HS = dims["NT"], dims["C"], dims["E"], dims["H"], dims["HS"]
    nc = bacc.Bacc(
        "TRN2", target_bir_lowering=False, debug=debug, enable_asserts=enable_asserts
    )
    aps = {
        "xT": nc.dram_tensor("xT", [C, NT], F32R, kind="ExternalInput").ap(),
        "xT32": nc.dram_tensor("xT32", [C, NT], F32, kind="ExternalInput").ap(),
        "rwT": nc.dram_tensor("rwT", [C, E], F32, kind="ExternalInput").ap(),
        "gate_up_w": nc.dram_tensor("gate_up_w", [E, C, 2 * H], F32R, kind="ExternalInput").ap(),
        "down_w": nc.dram_tensor("down_w", [E, H, C], F32R, kind="ExternalInput").ap(),
        "sgwT": nc.dram_tensor("sgwT", [C, HS], F32R, kind="ExternalInput").ap(),
        "suwT": nc.dram_tensor("suwT", [C, HS], F32R, kind="ExternalInput").ap(),
        "sdwT": nc.dram_tensor("sdwT", [HS, C], F32R, kind="ExternalInput").ap(),
        "y": nc.dram_tensor("y", [NT, C], F32, kind="ExternalOutput").ap(),
    }
    with tile.TileContext(nc) as tc:
        emit_moe(nc, tc, dims, aps)
    nc.compile()
    return nc


_NC_CACHE = {}
LAST_RESULTS = None


def kernel(**inputs):
    global LAST_RESULTS
    dims = {"NT": N_TOK // N_CORES, "C": C_FULL, "E": E_FULL, "H": H_FULL, "HS": HS_FULL}
    key = tuple(sorted(dims.items()))
    if key not in _NC_CACHE:
        _NC_CACHE[key] = build(dims)
    nc = _NC_CACHE[key]

    x = np.ascontiguousarray(np.asarray(inputs["x"], dtype=np.float32)).reshape(N_TOK, C_FULL)
    rwT = np.ascontiguousarray(np.asarray(inputs["router_w"], np.float32).T)
    guw = np.ascontiguousarray(np.asarray(inputs["gate_up_w"], np.float32))
    dw = np.ascontiguousarray(np.asarray(inputs["down_w"], np.float32))
    sgwT = np.ascontiguousarray(np.asarray(inputs["shared_gate_w"], np.float32).T)
    suwT = np.ascontiguousarray(np.asarray(inputs["shared_up_w"], np.float32).T)
    sdwT = np.ascontiguousarray(np.asarray(inputs["shared_down_w"], np.float32).T)

    NT = N_TOK // N_CORES
    in_maps = []
    for c in range(N_CORES):
        xT_c = np.ascontiguousarray(x[c * NT:(c + 1) * NT, :].T)
        in_maps.append({
            "xT": xT_c, "xT32": xT_c, "rwT": rwT, "gate_up_w": guw, "down_w": dw,
            "sgwT": sgwT, "suwT": suwT, "sdwT": sdwT,
        })

    trace = bool(os.environ.get("MOE_TRACE"))
    res = run_bass_kernel_spmd(nc, in_maps, list(range(N_CORES)), trace=trace)
    LAST_RESULTS = res
    y = np.concatenate([res.results[c]["y"] for c in range(N_CORES)], axis=0)
    return y.reshape(B, T, C_FULL).astype(np.float32)


# revision 15
# speedup vs baseline: 1.0115x; 1.0115x over previous
"""MoE transformer block (router top-2 + 8 experts + shared SwiGLU expert) on 8 trn2 cores.

Sharding: token-parallel. Each core gets 512 of the 4096 tokens and computes the
full mixture for its tokens (dense masked-combine over all 8 experts, identical
math to the reference). Weights are replicated; no collectives are needed.

Device kernel layout (per core, P=128 partitions):
  xT   [C, NT]  : tokens on the free dim, d_model on partitions (8 chunks of 128)
  gu_e = gate_up_w[e].T-free matmuls -> [2H partitions-chunks, NT free]
  h_e  = silu(gate) * up * combine_w  (combine broadcast via a K=1 ones matmul)
  y   += h_e.T-chunks @ down_w[e]     -> [NT partitions-chunks, C free]
  shared expert identically with host-pre-transposed weights.
All big matmuls use float32r (full-rate fp32 on the PE when free dim >= 256).
"""

import contextlib
import ctypes
import os
import sys
import types

sys.path.insert(0, "/opt/trn_rl_repo")

import numpy as np


def _install_ntff_shim():
    """Provide antenv.axon_hooks (missing in this image) so that
    run_bass_kernel_spmd(trace=True) can drive NTFF profiling through
    libaxon_pjrt.so's C ABI. Degrades to hook=None when the .so or its
    symbols are absent (bass_utils then skips tracing gracefully)."""
    if "antenv.axon_hooks" in sys.modules:
        return
    hook = None
    so_path = "/opt/axon/libaxon_pjrt.so"
    try:
        if os.path.exists(so_path):
            lib = ctypes.CDLL(so_path)
            if hasattr(lib, "axon_start_nrt_profile"):
                lib.axon_start_nrt_profile.argtypes = [
                    ctypes.POINTER(ctypes.c_int64),
                    ctypes.c_size_t,
                ]
                lib.axon_start_nrt_profile.restype = ctypes.c_int64
                lib.axon_stop_nrt_profile.argtypes = [ctypes.c_char_p]
                lib.axon_stop_nrt_profile.restype = ctypes.c_int64

                @contextlib.contextmanager
                def _hook(output_dir, device_ids):
                    import jax

                    jax.devices()
                    if device_ids:
                        ids = (ctypes.c_int64 * len(device_ids))(*device_ids)
                        rc = lib.axon_start_nrt_profile(ids, len(device_ids))
                    else:
                        rc = lib.axon_start_nrt_profile(None, 0)
                    if rc != 0:
                        raise RuntimeError(f"axon_start_nrt_profile rc={rc}")
                    try:
                        yield
                    finally:
                        n = lib.axon_stop_nrt_profile(str(output_dir).encode())
                        print(f"ntff profile: {n} file(s) -> {output_dir}", file=sys.stderr)

                hook = _hook
    except OSError:
        hook = None

    mod = types.ModuleType("antenv.axon_hooks")
    mod._hook = hook
    mod.get_axon_ntff_profile_hook = lambda: mod._hook

    def _set(h):
        mod._hook = h

    mod.set_axon_ntff_profile_hook = _set
    sys.modules["antenv.axon_hooks"] = mod


_install_ntff_shim()

import concourse.bass as bass
import concourse.mybir as mybir
import concourse.tile as tile
from concourse import bacc
from concourse.bass_utils import run_bass_kernel_spmd
from concourse.masks import make_identity

P = 128
F32 = mybir.dt.float32
F32R = mybir.dt.float32r
AF = mybir.ActivationFunctionType
ALU = mybir.AluOpType

# full problem dims
B, T, C_FULL = 4, 1024, 1024
E_FULL, H_FULL, HS_FULL = 8, 512, 2048
N_CORES = 8
N_TOK = B * T


def emit_moe(nc, tc, dims, aps):
    """Emit the per-core MoE kernel. dims: NT, C, E, H, HS. aps: dict of DRAM APs."""
    NT, C, E, H, HS = dims["NT"], dims["C"], dims["E"], dims["H"], dims["HS"]
    KC = C // P          # contraction chunks over d_model
    NCH = NT // P        # token chunks (tokens on partitions)
    JCH = 2 * H // P     # gate_up output chunks (0..JCH/2-1 gate, rest up)
    GCH = JCH // 2
    HCH = H // P         # expert hidden chunks
    HSCH = HS // P       # shared hidden chunks
    CW = min(512, C)     # matmul moving width for C-sized free dims
    CCH = C // CW
    NW = min(512, NT)    # moving width for token free dim
    assert NW == NT, "single token-span per core assumed"

    xT_d, rwT_d, guw_d, dw_d, sgwT_d, suwT_d, sdwT_d, y_d = (
        aps["xT"], aps["rwT"], aps["gate_up_w"], aps["down_w"],
        aps["sgwT"], aps["suwT"], aps["sdwT"], aps["y"],
    )
    xT32_d = aps["xT32"]

    # ---- pools ----
    import contextlib
    ctx = contextlib.ExitStack()

    # persistent tiles: one slot per tag in a bufs=1 pool
    res = ctx.enter_context(tc.tile_pool(name="res", bufs=1))
    xT_sb = res.tile([P, KC, NT], F32R, name="xt", tag="xt")
    y_sb = res.tile([P, NCH, C], F32, name="ysb", tag="ysb")
    comb_rows = res.tile([1, E, NT], F32, name="combt", tag="combt")
    rw_sb = res.tile([P, KC, E], F32, name="rwsb", tag="rwsb")
    ident = res.tile([P, P], F32, name="ident", tag="ident")

    make_identity(nc, ident)

    w1024 = ctx.enter_context(tc.tile_pool(name="w1024", bufs=9))    # guw + shared gate/up weight tiles
    dwp = ctx.enter_context(tc.tile_pool(name="dwp", bufs=5))        # down_w tiles
    sdwp = ctx.enter_context(tc.tile_pool(name="sdwp", bufs=4))      # shared down tiles
    wbp = ctx.enter_context(tc.tile_pool(name="wbp", bufs=2))        # combine broadcast tiles
    sgp = ctx.enter_context(tc.tile_pool(name="sgp", bufs=4))        # sigmoid tiles
    upp = ctx.enter_context(tc.tile_pool(name="upp", bufs=8))        # expert up/h tiles
    ssgp = ctx.enter_context(tc.tile_pool(name="ssgp", bufs=HSCH + 2))  # shared act tiles
    rsm = ctx.enter_context(tc.tile_pool(name="rsm", bufs=2))        # router small tiles
    rxp = ctx.enter_context(tc.tile_pool(name="rxp", bufs=2))        # fp32 x slices for router
    pgu = ctx.enter_context(tc.tile_pool(name="pgu", bufs=3, space="PSUM"))
    pdn = ctx.enter_context(tc.tile_pool(name="pdn", bufs=4, space="PSUM"))
    prt = ctx.enter_context(tc.tile_pool(name="prt", bufs=1, space="PSUM"))

    # ---- input DMAs: router weights first (small), then xT ----
    for k in range(KC):
        nc.sync.dma_start(out=rw_sb[:, k, :], in_=rwT_d[k * P:(k + 1) * P, :])

    # ---- router: logits -> top-2 mask -> sigmoid gates -> combT [E, NT] ----
    for n in range(NCH):
        lg_ps = prt.tile([P, E], F32, name="lg", tag="prt")
        # exact fp32 logits: top-2 selection must not flip on fp32r noise
        rx = rxp.tile([P, KC, P], F32, name="rx", tag="rx")
        for k in range(KC):
            nc.sync.dma_start(out=rx[:, k, :], in_=xT32_d[k * P:(k + 1) * P, n * P:(n + 1) * P])
        for k in range(KC):
            nc.tensor.matmul(
                lg_ps,
                lhsT=rx[:, k, :],
                rhs=rw_sb[:, k, :],
                start=(k == 0), stop=(k == KC - 1),
            )
        logits = rsm.tile([P, E], F32, name="logits", tag="lgs")
        nc.vector.tensor_copy(logits, lg_ps)
        m1 = rsm.tile([P, 1], F32, name="m1", tag="m1")
        nc.vector.reduce_max(m1, logits, axis=mybir.AxisListType.X)
        # (logits == m1) * -1e30  -> knock out the argmax
        eqb = rsm.tile([P, E], F32, name="eqb", tag="eqb")
        nc.vector.tensor_scalar(eqb, logits, m1, -1.0e30, op0=ALU.is_equal, op1=ALU.mult)
        masked = rsm.tile([P, E], F32, name="masked", tag="msk")
        nc.vector.tensor_add(masked, logits, eqb)
        m2 = rsm.tile([P, 1], F32, name="m2", tag="m2")
        nc.vector.reduce_max(m2, masked, axis=mybir.AxisListType.X)
        mask2 = rsm.tile([P, E], F32, name="mask2", tag="msk2")
        nc.vector.tensor_scalar(mask2, logits, m2, None, op0=ALU.is_ge)
        sig = rsm.tile([P, E], F32, name="sig", tag="sig")
        nc.scalar.activation(sig, logits, AF.Sigmoid)
        comb = rsm.tile([P, E], F32, name="comb", tag="comb")
        nc.vector.tensor_mul(comb, sig, mask2)
        # transpose each expert column [P, 1] -> a [1, P] row at partition 0
        for e in range(E):
            tp_ps = prt.tile([1, P], F32, name="tp", tag="prt")
            nc.tensor.transpose(tp_ps, comb[:, e:e + 1], ident)
            nc.vector.tensor_copy(comb_rows[0:1, e, n * P:(n + 1) * P], tp_ps)

    for k in range(KC):
        nc.sync.dma_start(out=xT_sb[:, k, :], in_=xT_d[k * P:(k + 1) * P, :])

    # ---- experts ----
    for e in range(E):
        # broadcast combine row e -> [P, NT]
        wb = wbp.tile([P, NT], F32, name="wb", tag="wb")
        nc.gpsimd.partition_broadcast(wb, comb_rows[0:1, e, :])

        guw_t = []
        for k in range(KC):
            g = w1024.tile([P, 2 * H], F32R, name=f"guw{k}", tag="w1024")
            nc.sync.dma_start(out=g, in_=guw_d[e, k * P:(k + 1) * P, :])
            guw_t.append(g)

        # process up chunks first, then gate chunks:
        #   h = silu(g) * (u*w) = (g * (u*w)) * sigmoid(g)
        up_t = []
        for j in list(range(GCH, JCH)) + list(range(GCH)):
            gu_ps = pgu.tile([P, NT], F32, name="gups", tag="pgu")
            for k in range(KC):
                nc.tensor.matmul(
                    gu_ps,
                    lhsT=guw_t[k][:, j * P:(j + 1) * P],
                    rhs=xT_sb[:, k, :],
                    start=(k == 0), stop=(k == KC - 1),
                )
            if j >= GCH:
                up = upp.tile([P, NT], F32R, name="up", tag="up")
                nc.vector.tensor_mul(up, gu_ps, wb)  # up * combine_w
                up_t.append(up)
            else:
                ut = up_t[j]
                nc.vector.tensor_mul(ut, gu_ps, ut)          # g * (u*w)
                sg = sgp.tile([P, NT], F32, name="sg", tag="sg")
                nc.scalar.activation(sg, gu_ps, AF.Sigmoid)  # sigmoid(g)
                nc.vector.tensor_mul(ut, sg, ut)             # h

        dw_t = []
        for k in range(HCH):
            d = dwp.tile([P, C], F32R, name=f"dw{k}", tag="dw")
            nc.sync.dma_start(out=d, in_=dw_d[e, k * P:(k + 1) * P, :])
            dw_t.append(d)
        for ch in range(CCH):
            for n in range(NCH):
                d_ps = pdn.tile([P, CW], F32, name="dps", tag="pdn")
                for k in range(HCH):
                    nc.tensor.matmul(
                        d_ps,
                        lhsT=up_t[k][:, n * P:(n + 1) * P],
                        rhs=dw_t[k][:, ch * CW:(ch + 1) * CW],
                        start=(k == 0), stop=(k == HCH - 1),
                    )
                ysl = y_sb[:, n, ch * CW:(ch + 1) * CW]
                if e == 0:
                    nc.vector.tensor_copy(ysl, d_ps)
                else:
                    nc.vector.tensor_add(ysl, ysl, d_ps)

    # ---- shared expert ----
    HS_BLK = min(8, HSCH)  # hs-chunk group per weight-tile residency
    ssg_t = [None] * HSCH
    for s, w_d in ((0, suwT_d), (1, sgwT_d)):  # up pass first, then gate pass
        for blk in range(0, HSCH, HS_BLK):
            sw_t = []
            for k in range(KC):
                w = w1024.tile([P, HS_BLK * P], F32R, name=f"sw{k}", tag="w1024")
                nc.sync.dma_start(
                    out=w, in_=w_d[k * P:(k + 1) * P, blk * P:(blk + HS_BLK) * P]
                )
                sw_t.append(w)
            for hsl in range(HS_BLK):
                hs = blk + hsl
                s_ps = pgu.tile([P, NT], F32, name="sps", tag="pgu")
                for k in range(KC):
                    nc.tensor.matmul(
                        s_ps,
                        lhsT=sw_t[k][:, hsl * P:(hsl + 1) * P],
                        rhs=xT_sb[:, k, :],
                        start=(k == 0), stop=(k == KC - 1),
                    )
                if s == 0:
                    ssg = ssgp.tile([P, NT], F32R, name="ssg", tag="ssg")
                    nc.vector.tensor_copy(ssg, s_ps)             # u
                    ssg_t[hs] = ssg
                else:
                    ut = ssg_t[hs]
                    nc.vector.tensor_mul(ut, s_ps, ut)           # g * u
                    ssig = sgp.tile([P, NT], F32, name="ssig", tag="sg")
                    nc.scalar.activation(ssig, s_ps, AF.Sigmoid)
                    nc.vector.tensor_mul(ut, ssig, ut)           # act

    for ch in range(CCH):
        d_ps = [pdn.tile([P, CW], F32, name=f"sdps{n}", tag="pdn") for n in range(NCH)]
        for k in range(HSCH):
            w = sdwp.tile([P, CW], F32R, name="sdw", tag="sdw")
            nc.sync.dma_start(
                out=w, in_=sdwT_d[k * P:(k + 1) * P, ch * CW:(ch + 1) * CW]
            )
            for n in range(NCH):
                nc.tensor.matmul(
                    d_ps[n],
                    lhsT=ssg_t[k][:, n * P:(n + 1) * P],
                    rhs=w,
                    start=(k == 0), stop=(k == HSCH - 1),
                )
        for n in range(NCH):
            ysl = y_sb[:, n, ch * CW:(ch + 1) * CW]
            nc.vector.tensor_add(ysl, ysl, d_ps[n])

    # ---- store ----
    for n in range(NCH):
        nc.sync.dma_start(out=y_d[n * P:(n + 1) * P, :], in_=y_sb[:, n, :])

    ctx.close()


def build(dims, debug=False, enable_asserts=False):
    NT, C, E, H, HS = dims["NT"], dims["C"], dims["E"], dims["H"], dims["HS"]
    nc = bacc.Bacc(
        "TRN2", target_bir_lowering=False, debug=debug, enable_asserts=enable_asserts
    )
    aps = {
        "xT": nc.dram_tensor("xT", [C, NT], F32R, kind="ExternalInput").ap(),
        "xT32": nc.dram_tensor("xT32", [C, NT], F32, kind="ExternalInput").ap(),
        "rwT": nc.dram_tensor("rwT", [C, E], F32, kind="ExternalInput").ap(),
        "gate_up_w": nc.dram_tensor("gate_up_w", [E, C, 2 * H], F32R, kind="ExternalInput").ap(),
        "down_w": nc.dram_tensor("down_w", [E, H, C], F32R, kind="ExternalInput").ap(),
        "sgwT": nc.dram_tensor("sgwT", [C, HS], F32R, kind="ExternalInput").ap(),
        "suwT": nc.dram_tensor("suwT", [C, HS], F32R, kind="ExternalInput").ap(),
        "sdwT": nc.dram_tensor("sdwT", [HS, C], F32R, kind="ExternalInput").ap(),
        "y": nc.dram_tensor("y", [NT, C], F32, kind="ExternalOutput").ap(),
    }
    with tile.TileContext(nc) as tc:
        emit_moe(nc, tc, dims, aps)
    nc.compile()
    return nc


_NC_CACHE = {}
LAST_RESULTS = None


def kernel(**inputs):
    global LAST_RESULTS
    dims = {"NT": N_TOK // N_CORES, "C": C_FULL, "E": E_FULL, "H": H_FULL, "HS": HS_FULL}
    key = tuple(sorted(dims.items()))
    if key not in _NC_CACHE:
        _NC_CACHE[key] = build(dims)
    nc = _NC_CACHE[key]

    x = np.ascontiguousarray(np.asarray(inputs["x"], dtype=np.float32)).reshape(N_TOK, C_FULL)
    rwT = np.ascontiguousarray(np.asarray(inputs["router_w"], np.float32).T)
    guw = np.ascontiguousarray(np.asarray(inputs["gate_up_w"], np.float32))
    dw = np.ascontiguousarray(np.asarray(inputs["down_w"], np.float32))
    sgwT = np.ascontiguousarray(np.asarray(inputs["shared_gate_w"], np.float32).T)
    suwT = np.ascontiguousarray(np.asarray(inputs["shared_up_w"], np.float32).T)
    sdwT = np.ascontiguousarray(np.asarray(inputs["shared_down_w"], np.float32).T)

    NT = N_TOK // N_CORES
    in_maps = []
    for c in range(N_CORES):
        xT_c = np.ascontiguousarray(x[c * NT:(c + 1) * NT, :].T)
        in_maps.append({
            "xT": xT_c, "xT32": xT_c, "rwT": rwT, "gate_up_w": guw, "down_w": dw,
            "sgwT": sgwT, "suwT": suwT, "sdwT": sdwT,
        })

    trace = bool(os.environ.get("MOE_TRACE"))
    res = run_bass_kernel_spmd(nc, in_maps, list(range(N_CORES)), trace=trace)
    LAST_RESULTS = res
    y = np.concatenate([res.results[c]["y"] for c in range(N_CORES)], axis=0)
    return y.reshape(B, T, C_FULL).astype(np.float32)


# revision 16
# speedup vs baseline: 1.0577x; 1.0457x over previous
"""MoE transformer block (router top-2 + 8 experts + shared SwiGLU expert) on 8 trn2 cores.

Sharding: token-parallel. Each core gets 512 of the 4096 tokens and computes the
full mixture for its tokens (dense masked-combine over all 8 experts, identical
math to the reference). Weights are replicated; no collectives are needed.

Device kernel layout (per core, P=128 partitions):
  xT   [C, NT]  : tokens on the free dim, d_model on partitions (8 chunks of 128)
  gu_e = gate_up_w[e].T-free matmuls -> [2H partitions-chunks, NT free]
  h_e  = silu(gate) * up * combine_w  (combine broadcast via a K=1 ones matmul)
  y   += h_e.T-chunks @ down_w[e]     -> [NT partitions-chunks, C free]
  shared expert identically with host-pre-transposed weights.
All big matmuls use float32r (full-rate fp32 on the PE when free dim >= 256).
"""

import contextlib
import ctypes
import os
import sys
import types

sys.path.insert(0, "/opt/trn_rl_repo")

import numpy as np


def _install_ntff_shim():
    """Provide antenv.axon_hooks (missing in this image) so that
    run_bass_kernel_spmd(trace=True) can drive NTFF profiling through
    libaxon_pjrt.so's C ABI. Degrades to hook=None when the .so or its
    symbols are absent (bass_utils then skips tracing gracefully)."""
    if "antenv.axon_hooks" in sys.modules:
        return
    hook = None
    so_path = "/opt/axon/libaxon_pjrt.so"
    try:
        if os.path.exists(so_path):
            lib = ctypes.CDLL(so_path)
            if hasattr(lib, "axon_start_nrt_profile"):
                lib.axon_start_nrt_profile.argtypes = [
                    ctypes.POINTER(ctypes.c_int64),
                    ctypes.c_size_t,
                ]
                lib.axon_start_nrt_profile.restype = ctypes.c_int64
                lib.axon_stop_nrt_profile.argtypes = [ctypes.c_char_p]
                lib.axon_stop_nrt_profile.restype = ctypes.c_int64

                @contextlib.contextmanager
                def _hook(output_dir, device_ids):
                    import jax

                    jax.devices()
                    if device_ids:
                        ids = (ctypes.c_int64 * len(device_ids))(*device_ids)
                        rc = lib.axon_start_nrt_profile(ids, len(device_ids))
                    else:
                        rc = lib.axon_start_nrt_profile(None, 0)
                    if rc != 0:
                        raise RuntimeError(f"axon_start_nrt_profile rc={rc}")
                    try:
                        yield
                    finally:
                        n = lib.axon_stop_nrt_profile(str(output_dir).encode())
                        print(f"ntff profile: {n} file(s) -> {output_dir}", file=sys.stderr)

                hook = _hook
    except OSError:
        hook = None

    mod = types.ModuleType("antenv.axon_hooks")
    mod._hook = hook
    mod.get_axon_ntff_profile_hook = lambda: mod._hook

    def _set(h):
        mod._hook = h

    mod.set_axon_ntff_profile_hook = _set
    sys.modules["antenv.axon_hooks"] = mod


_install_ntff_shim()

import concourse.bass as bass
import concourse.mybir as mybir
import concourse.tile as tile
from concourse import bacc
from concourse.bass_utils import run_bass_kernel_spmd
from concourse.masks import make_identity

P = 128
F32 = mybir.dt.float32
F32R = mybir.dt.float32r
AF = mybir.ActivationFunctionType
ALU = mybir.AluOpType

# full problem dims
B, T, C_FULL = 4, 1024, 1024
E_FULL, H_FULL, HS_FULL = 8, 512, 2048
N_CORES = 8
N_TOK = B * T


def emit_moe(nc, tc, dims, aps):
    """Emit the per-core MoE kernel. dims: NT, C, E, H, HS. aps: dict of DRAM APs."""
    NT, C, E, H, HS = dims["NT"], dims["C"], dims["E"], dims["H"], dims["HS"]
    KC = C // P          # contraction chunks over d_model
    NCH = NT // P        # token chunks (tokens on partitions)
    JCH = 2 * H // P     # gate_up output chunks (0..JCH/2-1 gate, rest up)
    GCH = JCH // 2
    HCH = H // P         # expert hidden chunks
    HSCH = HS // P       # shared hidden chunks
    CW = min(512, C)     # matmul moving width for C-sized free dims
    CCH = C // CW
    NW = min(512, NT)    # moving width for token free dim
    assert NW == NT, "single token-span per core assumed"

    xT_d, rwT_d, guw_d, dw_d, sgwT_d, suwT_d, sdwT_d, y_d = (
        aps["xT"], aps["rwT"], aps["gate_up_w"], aps["down_w"],
        aps["sgwT"], aps["suwT"], aps["sdwT"], aps["y"],
    )
    xT32_d = aps["xT32"]

    # ---- pools ----
    import contextlib
    ctx = contextlib.ExitStack()

    # persistent tiles: one slot per tag in a bufs=1 pool
    res = ctx.enter_context(tc.tile_pool(name="res", bufs=1))
    xT_sb = res.tile([P, KC, NT], F32R, name="xt", tag="xt")
    y_sb = res.tile([P, NCH, C], F32, name="ysb", tag="ysb")
    comb_rows = res.tile([1, E, NT], F32, name="combt", tag="combt")
    rw_sb = res.tile([P, KC, E], F32, name="rwsb", tag="rwsb")
    ident = res.tile([P, P], F32, name="ident", tag="ident")

    make_identity(nc, ident)

    w1024 = ctx.enter_context(tc.tile_pool(name="w1024", bufs=12))    # guw + shared gate/up weight tiles
    dwp = ctx.enter_context(tc.tile_pool(name="dwp", bufs=6))        # down_w tiles
    sdwp = ctx.enter_context(tc.tile_pool(name="sdwp", bufs=6))      # shared down tiles
    wbp = ctx.enter_context(tc.tile_pool(name="wbp", bufs=2))        # combine broadcast tiles
    sgp = ctx.enter_context(tc.tile_pool(name="sgp", bufs=4))        # sigmoid tiles
    upp = ctx.enter_context(tc.tile_pool(name="upp", bufs=8))        # expert up/h tiles
    ssgp = ctx.enter_context(tc.tile_pool(name="ssgp", bufs=HSCH + 2))  # shared act tiles
    rsm = ctx.enter_context(tc.tile_pool(name="rsm", bufs=2))        # router small tiles
    rxp = ctx.enter_context(tc.tile_pool(name="rxp", bufs=2))        # fp32 x slices for router
    pgu = ctx.enter_context(tc.tile_pool(name="pgu", bufs=3, space="PSUM"))
    pdn = ctx.enter_context(tc.tile_pool(name="pdn", bufs=4, space="PSUM"))
    prt = ctx.enter_context(tc.tile_pool(name="prt", bufs=1, space="PSUM"))

    # ---- input DMAs: router weights first (small), then xT ----
    for k in range(KC):
        nc.sync.dma_start(out=rw_sb[:, k, :], in_=rwT_d[k * P:(k + 1) * P, :])

    # ---- router: logits -> top-2 mask -> sigmoid gates -> combT [E, NT] ----
    for n in range(NCH):
        lg_ps = prt.tile([P, E], F32, name="lg", tag="prt")
        # exact fp32 logits: top-2 selection must not flip on fp32r noise
        rx = rxp.tile([P, KC, P], F32, name="rx", tag="rx")
        for k in range(KC):
            nc.sync.dma_start(out=rx[:, k, :], in_=xT32_d[k * P:(k + 1) * P, n * P:(n + 1) * P])
        for k in range(KC):
            nc.tensor.matmul(
                lg_ps,
                lhsT=rx[:, k, :],
                rhs=rw_sb[:, k, :],
                start=(k == 0), stop=(k == KC - 1),
            )
        logits = rsm.tile([P, E], F32, name="logits", tag="lgs")
        nc.vector.tensor_copy(logits, lg_ps)
        m1 = rsm.tile([P, 1], F32, name="m1", tag="m1")
        nc.vector.reduce_max(m1, logits, axis=mybir.AxisListType.X)
        # (logits == m1) * -1e30  -> knock out the argmax
        eqb = rsm.tile([P, E], F32, name="eqb", tag="eqb")
        nc.vector.tensor_scalar(eqb, logits, m1, -1.0e30, op0=ALU.is_equal, op1=ALU.mult)
        masked = rsm.tile([P, E], F32, name="masked", tag="msk")
        nc.vector.tensor_add(masked, logits, eqb)
        m2 = rsm.tile([P, 1], F32, name="m2", tag="m2")
        nc.vector.reduce_max(m2, masked, axis=mybir.AxisListType.X)
        mask2 = rsm.tile([P, E], F32, name="mask2", tag="msk2")
        nc.vector.tensor_scalar(mask2, logits, m2, None, op0=ALU.is_ge)
        sig = rsm.tile([P, E], F32, name="sig", tag="sig")
        nc.scalar.activation(sig, logits, AF.Sigmoid)
        comb = rsm.tile([P, E], F32, name="comb", tag="comb")
        nc.vector.tensor_mul(comb, sig, mask2)
        # transpose each expert column [P, 1] -> a [1, P] row at partition 0
        for e in range(E):
            tp_ps = prt.tile([1, P], F32, name="tp", tag="prt")
            nc.tensor.transpose(tp_ps, comb[:, e:e + 1], ident)
            nc.vector.tensor_copy(comb_rows[0:1, e, n * P:(n + 1) * P], tp_ps)

    for k in range(KC):
        nc.sync.dma_start(out=xT_sb[:, k, :], in_=xT_d[k * P:(k + 1) * P, :])

    # ---- experts ----
    for e in range(E):
        # broadcast combine row e -> [P, NT]
        wb = wbp.tile([P, NT], F32, name="wb", tag="wb")
        nc.gpsimd.partition_broadcast(wb, comb_rows[0:1, e, :])

        guw_t = []
        for k in range(KC):
            g = w1024.tile([P, 2 * H], F32R, name=f"guw{k}", tag="w1024")
            nc.sync.dma_start(out=g, in_=guw_d[e, k * P:(k + 1) * P, :])
            guw_t.append(g)

        # process up chunks first, then gate chunks:
        #   h = silu(g) * (u*w) = (g * (u*w)) * sigmoid(g)
        up_t = []
        for j in list(range(GCH, JCH)) + list(range(GCH)):
            gu_ps = pgu.tile([P, NT], F32, name="gups", tag="pgu")
            for k in range(KC):
                nc.tensor.matmul(
                    gu_ps,
                    lhsT=guw_t[k][:, j * P:(j + 1) * P],
                    rhs=xT_sb[:, k, :],
                    start=(k == 0), stop=(k == KC - 1),
                )
            if j >= GCH:
                up = upp.tile([P, NT], F32R, name="up", tag="up")
                nc.vector.tensor_mul(up, gu_ps, wb)  # up * combine_w
                up_t.append(up)
            else:
                ut = up_t[j]
                nc.vector.tensor_mul(ut, gu_ps, ut)          # g * (u*w)
                sg = sgp.tile([P, NT], F32, name="sg", tag="sg")
                nc.scalar.activation(sg, gu_ps, AF.Sigmoid)  # sigmoid(g)
                nc.vector.tensor_mul(ut, sg, ut)             # h

        dw_t = []
        for k in range(HCH):
            d = dwp.tile([P, C], F32R, name=f"dw{k}", tag="dw")
            nc.sync.dma_start(out=d, in_=dw_d[e, k * P:(k + 1) * P, :])
            dw_t.append(d)
        for ch in range(CCH):
            for n in range(NCH):
                d_ps = pdn.tile([P, CW], F32, name="dps", tag="pdn")
                for k in range(HCH):
                    nc.tensor.matmul(
                        d_ps,
                        lhsT=up_t[k][:, n * P:(n + 1) * P],
                        rhs=dw_t[k][:, ch * CW:(ch + 1) * CW],
                        start=(k == 0), stop=(k == HCH - 1),
                    )
                ysl = y_sb[:, n, ch * CW:(ch + 1) * CW]
                if e == 0:
                    nc.vector.tensor_copy(ysl, d_ps)
                else:
                    nc.vector.tensor_add(ysl, ysl, d_ps)

    # ---- shared expert ----
    HS_BLK = min(4, HSCH)  # hs-chunk group per weight-tile residency
    ssg_t = [None] * HSCH
    for s, w_d in ((0, suwT_d), (1, sgwT_d)):  # up pass first, then gate pass
        for blk in range(0, HSCH, HS_BLK):
            sw_t = []
            for k in range(KC):
                w = w1024.tile([P, HS_BLK * P], F32R, name=f"sw{k}", tag="w1024")
                nc.sync.dma_start(
                    out=w, in_=w_d[k * P:(k + 1) * P, blk * P:(blk + HS_BLK) * P]
                )
                sw_t.append(w)
            for hsl in range(HS_BLK):
                hs = blk + hsl
                s_ps = pgu.tile([P, NT], F32, name="sps", tag="pgu")
                for k in range(KC):
                    nc.tensor.matmul(
                        s_ps,
                        lhsT=sw_t[k][:, hsl * P:(hsl + 1) * P],
                        rhs=xT_sb[:, k, :],
                        start=(k == 0), stop=(k == KC - 1),
                    )
                if s == 0:
                    ssg = ssgp.tile([P, NT], F32R, name="ssg", tag="ssg")
                    nc.vector.tensor_copy(ssg, s_ps)             # u
                    ssg_t[hs] = ssg
                else:
                    ut = ssg_t[hs]
                    nc.vector.tensor_mul(ut, s_ps, ut)           # g * u
                    ssig = sgp.tile([P, NT], F32, name="ssig", tag="sg")
                    nc.scalar.activation(ssig, s_ps, AF.Sigmoid)
                    nc.vector.tensor_mul(ut, ssig, ut)           # act

    for ch in range(CCH):
        d_ps = [pdn.tile([P, CW], F32, name=f"sdps{n}", tag="pdn") for n in range(NCH)]
        for k in range(HSCH):
            w = sdwp.tile([P, CW], F32R, name="sdw", tag="sdw")
            nc.sync.dma_start(
                out=w, in_=sdwT_d[k * P:(k + 1) * P, ch * CW:(ch + 1) * CW]
            )
            for n in range(NCH):
                nc.tensor.matmul(
                    d_ps[n],
                    lhsT=ssg_t[k][:, n * P:(n + 1) * P],
                    rhs=w,
                    start=(k == 0), stop=(k == HSCH - 1),
                )
        for n in range(NCH):
            ysl = y_sb[:, n, ch * CW:(ch + 1) * CW]
            nc.vector.tensor_add(ysl, ysl, d_ps[n])
            if ch == CCH - 1:
                nc.sync.dma_start(out=y_d[n * P:(n + 1) * P, :], in_=y_sb[:, n, :])

    ctx.close()


def build(dims, debug=False, enable_asserts=False):
    NT, C, E, H, HS = dims["NT"], dims["C"], dims["E"], dims["H"], dims["HS"]
    nc = bacc.Bacc(
        "TRN2", target_bir_lowering=False, debug=debug, enable_asserts=enable_asserts
    )
    aps = {
        "xT": nc.dram_tensor("xT", [C, NT], F32R, kind="ExternalInput").ap(),
        "xT32": nc.dram_tensor("xT32", [C, NT], F32, kind="ExternalInput").ap(),
        "rwT": nc.dram_tensor("rwT", [C, E], F32, kind="ExternalInput").ap(),
        "gate_up_w": nc.dram_tensor("gate_up_w", [E, C, 2 * H], F32R, kind="ExternalInput").ap(),
        "down_w": nc.dram_tensor("down_w", [E, H, C], F32R, kind="ExternalInput").ap(),
        "sgwT": nc.dram_tensor("sgwT", [C, HS], F32R, kind="ExternalInput").ap(),
        "suwT": nc.dram_tensor("suwT", [C, HS], F32R, kind="ExternalInput").ap(),
        "sdwT": nc.dram_tensor("sdwT", [HS, C], F32R, kind="ExternalInput").ap(),
        "y": nc.dram_tensor("y", [NT, C], F32, kind="ExternalOutput").ap(),
    }
    with tile.TileContext(nc) as tc:
        emit_moe(nc, tc, dims, aps)
    nc.compile()
    return nc


_NC_CACHE = {}
LAST_RESULTS = None


def kernel(**inputs):
    global LAST_RESULTS
    dims = {"NT": N_TOK // N_CORES, "C": C_FULL, "E": E_FULL, "H": H_FULL, "HS": HS_FULL}
    key = tuple(sorted(dims.items()))
    if key not in _NC_CACHE:
        _NC_CACHE[key] = build(dims)
    nc = _NC_CACHE[key]

    x = np.ascontiguousarray(np.asarray(inputs["x"], dtype=np.float32)).reshape(N_TOK, C_FULL)
    rwT = np.ascontiguousarray(np.asarray(inputs["router_w"], np.float32).T)
    guw = np.ascontiguousarray(np.asarray(inputs["gate_up_w"], np.float32))
    dw = np.ascontiguousarray(np.asarray(inputs["down_w"], np.float32))
    sgwT = np.ascontiguousarray(np.asarray(inputs["shared_gate_w"], np.float32).T)
    suwT = np.ascontiguousarray(np.asarray(inputs["shared_up_w"], np.float32).T)
    sdwT = np.ascontiguousarray(np.asarray(inputs["shared_down_w"], np.float32).T)

    NT = N_TOK // N_CORES
    in_maps = []
    for c in range(N_CORES):
        xT_c = np.ascontiguousarray(x[c * NT:(c + 1) * NT, :].T)
        in_maps.append({
            "xT": xT_c, "xT32": xT_c, "rwT": rwT, "gate_up_w": guw, "down_w": dw,
            "sgwT": sgwT, "suwT": suwT, "sdwT": sdwT,
        })

    trace = bool(os.environ.get("MOE_TRACE"))
    res = run_bass_kernel_spmd(nc, in_maps, list(range(N_CORES)), trace=trace)
    LAST_RESULTS = res
    y = np.concatenate([res.results[c]["y"] for c in range(N_CORES)], axis=0)
    return y.reshape(B, T, C_FULL).astype(np.float32)


# revision 17
# speedup vs baseline: 1.1273x; 1.0658x over previous
"""MoE transformer block (router top-2 + 8 experts + shared SwiGLU expert) on 8 trn2 cores.

Sharding: token-parallel. Each core gets 512 of the 4096 tokens and computes the
full mixture for its tokens (dense masked-combine over all 8 experts, identical
math to the reference). Weights are replicated; no collectives are needed.

Device kernel layout (per core, P=128 partitions):
  xT   [C, NT]  : tokens on the free dim, d_model on partitions (8 chunks of 128)
  gu_e = gate_up_w[e].T-free matmuls -> [2H partitions-chunks, NT free]
  h_e  = silu(gate) * up * combine_w  (combine broadcast via a K=1 ones matmul)
  y   += h_e.T-chunks @ down_w[e]     -> [NT partitions-chunks, C free]
  shared expert identically with host-pre-transposed weights.
All big matmuls use float32r (full-rate fp32 on the PE when free dim >= 256).
"""

import contextlib
import ctypes
import os
import sys
import types

sys.path.insert(0, "/opt/trn_rl_repo")

import numpy as np


def _install_ntff_shim():
    """Provide antenv.axon_hooks (missing in this image) so that
    run_bass_kernel_spmd(trace=True) can drive NTFF profiling through
    libaxon_pjrt.so's C ABI. Degrades to hook=None when the .so or its
    symbols are absent (bass_utils then skips tracing gracefully)."""
    if "antenv.axon_hooks" in sys.modules:
        return
    hook = None
    so_path = "/opt/axon/libaxon_pjrt.so"
    try:
        if os.path.exists(so_path):
            lib = ctypes.CDLL(so_path)
            if hasattr(lib, "axon_start_nrt_profile"):
                lib.axon_start_nrt_profile.argtypes = [
                    ctypes.POINTER(ctypes.c_int64),
                    ctypes.c_size_t,
                ]
                lib.axon_start_nrt_profile.restype = ctypes.c_int64
                lib.axon_stop_nrt_profile.argtypes = [ctypes.c_char_p]
                lib.axon_stop_nrt_profile.restype = ctypes.c_int64

                @contextlib.contextmanager
                def _hook(output_dir, device_ids):
                    import jax

                    jax.devices()
                    if device_ids:
                        ids = (ctypes.c_int64 * len(device_ids))(*device_ids)
                        rc = lib.axon_start_nrt_profile(ids, len(device_ids))
                    else:
                        rc = lib.axon_start_nrt_profile(None, 0)
                    if rc != 0:
                        raise RuntimeError(f"axon_start_nrt_profile rc={rc}")
                    try:
                        yield
                    finally:
                        n = lib.axon_stop_nrt_profile(str(output_dir).encode())
                        print(f"ntff profile: {n} file(s) -> {output_dir}", file=sys.stderr)

                hook = _hook
    except OSError:
        hook = None

    mod = types.ModuleType("antenv.axon_hooks")
    mod._hook = hook
    mod.get_axon_ntff_profile_hook = lambda: mod._hook

    def _set(h):
        mod._hook = h

    mod.set_axon_ntff_profile_hook = _set
    sys.modules["antenv.axon_hooks"] = mod


_install_ntff_shim()

import concourse.bass as bass
import concourse.mybir as mybir
import concourse.tile as tile
from concourse import bacc
from concourse.bass_utils import run_bass_kernel_spmd
from concourse.masks import make_identity

P = 128
F32 = mybir.dt.float32
F32R = mybir.dt.float32r
AF = mybir.ActivationFunctionType
ALU = mybir.AluOpType

# full problem dims
B, T, C_FULL = 4, 1024, 1024
E_FULL, H_FULL, HS_FULL = 8, 512, 2048
N_CORES = 8
N_TOK = B * T


def emit_moe(nc, tc, dims, aps):
    """Emit the per-core MoE kernel. dims: NT, C, E, H, HS. aps: dict of DRAM APs."""
    NT, C, E, H, HS = dims["NT"], dims["C"], dims["E"], dims["H"], dims["HS"]
    KC = C // P          # contraction chunks over d_model
    NCH = NT // P        # token chunks (tokens on partitions)
    JCH = 2 * H // P     # gate_up output chunks (0..JCH/2-1 gate, rest up)
    GCH = JCH // 2
    HCH = H // P         # expert hidden chunks
    HSCH = HS // P       # shared hidden chunks
    CW = min(512, C)     # matmul moving width for C-sized free dims
    CCH = C // CW
    NW = min(512, NT)    # moving width for token free dim
    assert NW == NT, "single token-span per core assumed"

    xT_d, rwT_d, guw_d, dw_d, sgwT_d, suwT_d, sdwT_d, y_d = (
        aps["xT"], aps["rwT"], aps["gate_up_w"], aps["down_w"],
        aps["sgwT"], aps["suwT"], aps["sdwT"], aps["y"],
    )
    xT32_d = aps["xT32"]

    # ---- pools ----
    import contextlib
    ctx = contextlib.ExitStack()

    # persistent tiles: one slot per tag in a bufs=1 pool
    res = ctx.enter_context(tc.tile_pool(name="res", bufs=1))
    xT_sb = res.tile([P, KC, NT], F32R, name="xt", tag="xt")
    y_sb = res.tile([P, NCH, C], F32, name="ysb", tag="ysb")
    comb_rows = res.tile([1, E, NT], F32, name="combt", tag="combt")
    rw_sb = res.tile([P, KC, E], F32, name="rwsb", tag="rwsb")
    ident = res.tile([P, P], F32, name="ident", tag="ident")

    make_identity(nc, ident)

    w1024 = ctx.enter_context(tc.tile_pool(name="w1024", bufs=13))    # guw + shared gate/up weight tiles
    dwp = ctx.enter_context(tc.tile_pool(name="dwp", bufs=6))        # down_w tiles
    sdwp = ctx.enter_context(tc.tile_pool(name="sdwp", bufs=6))      # shared down tiles
    wbp = ctx.enter_context(tc.tile_pool(name="wbp", bufs=2))        # combine broadcast tiles
    sgp = ctx.enter_context(tc.tile_pool(name="sgp", bufs=3))        # sigmoid tiles
    upp = ctx.enter_context(tc.tile_pool(name="upp", bufs=7))        # expert up/h tiles
    ssgp = ctx.enter_context(tc.tile_pool(name="ssgp", bufs=HSCH + 2))  # shared act tiles
    rsm = ctx.enter_context(tc.tile_pool(name="rsm", bufs=2))        # router small tiles
    rxp = ctx.enter_context(tc.tile_pool(name="rxp", bufs=2))        # fp32 x slices for router
    pgu = ctx.enter_context(tc.tile_pool(name="pgu", bufs=4, space="PSUM"))
    pdn = ctx.enter_context(tc.tile_pool(name="pdn", bufs=4, space="PSUM"))

    # ---- input DMAs: router weights first (small), then xT ----
    for k in range(KC):
        nc.sync.dma_start(out=rw_sb[:, k, :], in_=rwT_d[k * P:(k + 1) * P, :])

    # ---- router: logits -> top-2 mask -> sigmoid gates -> combT [E, NT] ----
    comb_t = []
    for n in range(NCH):
        lg_ps = pgu.tile([P, E], F32, name="lg", tag="pgu")
        # exact fp32 logits: top-2 selection must not flip on fp32r noise
        rx = rxp.tile([P, KC, P], F32, name="rx", tag="rx")
        for k in range(KC):
            nc.sync.dma_start(out=rx[:, k, :], in_=xT32_d[k * P:(k + 1) * P, n * P:(n + 1) * P])
        for k in range(KC):
            nc.tensor.matmul(
                lg_ps,
                lhsT=rx[:, k, :],
                rhs=rw_sb[:, k, :],
                start=(k == 0), stop=(k == KC - 1),
            )
        logits = rsm.tile([P, E], F32, name="logits", tag="lgs")
        nc.vector.tensor_copy(logits, lg_ps)
        m1 = rsm.tile([P, 1], F32, name="m1", tag="m1")
        nc.vector.reduce_max(m1, logits, axis=mybir.AxisListType.X)
        # (logits == m1) * -1e30  -> knock out the argmax
        eqb = rsm.tile([P, E], F32, name="eqb", tag="eqb")
        nc.vector.tensor_scalar(eqb, logits, m1, -1.0e30, op0=ALU.is_equal, op1=ALU.mult)
        masked = rsm.tile([P, E], F32, name="masked", tag="msk")
        nc.vector.tensor_add(masked, logits, eqb)
        m2 = rsm.tile([P, 1], F32, name="m2", tag="m2")
        nc.vector.reduce_max(m2, masked, axis=mybir.AxisListType.X)
        mask2 = rsm.tile([P, E], F32, name="mask2", tag="msk2")
        nc.vector.tensor_scalar(mask2, logits, m2, None, op0=ALU.is_ge)
        sig = rsm.tile([P, E], F32, name="sig", tag="sig")
        nc.scalar.activation(sig, logits, AF.Sigmoid)
        comb = rsm.tile([P, E], F32, name="comb", tag="comb", bufs=NCH + 1)
        nc.vector.tensor_mul(comb, sig, mask2)
        comb_t.append(comb)

    # transpose pass: each expert column [P, 1] -> a [1, P] row at partition 0
    for n in range(NCH):
        for e in range(E):
            tp_ps = pgu.tile([1, P], F32, name="tp", tag="pgu")
            nc.tensor.transpose(tp_ps, comb_t[n][:, e:e + 1], ident)
            nc.vector.tensor_copy(comb_rows[0:1, e, n * P:(n + 1) * P], tp_ps)

    for k in range(KC):
        nc.sync.dma_start(out=xT_sb[:, k, :], in_=xT_d[k * P:(k + 1) * P, :])

    # ---- experts ----
    for e in range(E):
        # broadcast combine row e -> [P, NT]
        wb = wbp.tile([P, NT], F32, name="wb", tag="wb")
        nc.gpsimd.partition_broadcast(wb, comb_rows[0:1, e, :])

        guw_t = []
        for k in range(KC):
            g = w1024.tile([P, 2 * H], F32R, name=f"guw{k}", tag="w1024")
            nc.sync.dma_start(out=g, in_=guw_d[e, k * P:(k + 1) * P, :])
            guw_t.append(g)

        # process up chunks first, then gate chunks:
        #   h = silu(g) * (u*w) = (g * (u*w)) * sigmoid(g)
        up_t = []
        for j in list(range(GCH, JCH)) + list(range(GCH)):
            gu_ps = pgu.tile([P, NT], F32, name="gups", tag="pgu")
            for k in range(KC):
                nc.tensor.matmul(
                    gu_ps,
                    lhsT=guw_t[k][:, j * P:(j + 1) * P],
                    rhs=xT_sb[:, k, :],
                    start=(k == 0), stop=(k == KC - 1),
                )
            if j >= GCH:
                up = upp.tile([P, NT], F32R, name="up", tag="up")
                nc.vector.tensor_mul(up, gu_ps, wb)  # up * combine_w
                up_t.append(up)
            else:
                ut = up_t[j]
                nc.vector.tensor_mul(ut, gu_ps, ut)          # g * (u*w)
                sg = sgp.tile([P, NT], F32, name="sg", tag="sg")
                nc.scalar.activation(sg, gu_ps, AF.Sigmoid)  # sigmoid(g)
                nc.vector.tensor_mul(ut, sg, ut)             # h

        dw_t = []
        for k in range(HCH):
            d = dwp.tile([P, C], F32R, name=f"dw{k}", tag="dw")
            nc.sync.dma_start(out=d, in_=dw_d[e, k * P:(k + 1) * P, :])
            dw_t.append(d)
        for ch in range(CCH):
            for n in range(NCH):
                d_ps = pdn.tile([P, CW], F32, name="dps", tag="pdn")
                for k in range(HCH):
                    nc.tensor.matmul(
                        d_ps,
                        lhsT=up_t[k][:, n * P:(n + 1) * P],
                        rhs=dw_t[k][:, ch * CW:(ch + 1) * CW],
                        start=(k == 0), stop=(k == HCH - 1),
                    )
                ysl = y_sb[:, n, ch * CW:(ch + 1) * CW]
                if e == 0:
                    nc.vector.tensor_copy(ysl, d_ps)
                else:
                    nc.vector.tensor_add(ysl, ysl, d_ps)

    # ---- shared expert ----
    HS_BLK = min(4, HSCH)  # hs-chunk group per weight-tile residency
    ssg_t = [None] * HSCH
    for s, w_d in ((0, suwT_d), (1, sgwT_d)):  # up pass first, then gate pass
        for blk in range(0, HSCH, HS_BLK):
            sw_t = []
            for k in range(KC):
                w = w1024.tile([P, HS_BLK * P], F32R, name=f"sw{k}", tag="w1024")
                nc.sync.dma_start(
                    out=w, in_=w_d[k * P:(k + 1) * P, blk * P:(blk + HS_BLK) * P]
                )
                sw_t.append(w)
            for hsl in range(HS_BLK):
                hs = blk + hsl
                s_ps = pgu.tile([P, NT], F32, name="sps", tag="pgu")
                for k in range(KC):
                    nc.tensor.matmul(
                        s_ps,
                        lhsT=sw_t[k][:, hsl * P:(hsl + 1) * P],
                        rhs=xT_sb[:, k, :],
                        start=(k == 0), stop=(k == KC - 1),
                    )
                if s == 0:
                    ssg = ssgp.tile([P, NT], F32R, name="ssg", tag="ssg")
                    nc.vector.tensor_copy(ssg, s_ps)             # u
                    ssg_t[hs] = ssg
                else:
                    ut = ssg_t[hs]
                    nc.vector.tensor_mul(ut, s_ps, ut)           # g * u
                    ssig = sgp.tile([P, NT], F32, name="ssig", tag="sg")
                    nc.scalar.activation(ssig, s_ps, AF.Sigmoid)
                    nc.vector.tensor_mul(ut, ssig, ut)           # act

    for ch in range(CCH):
        d_ps = [pdn.tile([P, CW], F32, name=f"sdps{n}", tag="pdn") for n in range(NCH)]
        for k in range(HSCH):
            w = sdwp.tile([P, CW], F32R, name="sdw", tag="sdw")
            nc.sync.dma_start(
                out=w, in_=sdwT_d[k * P:(k + 1) * P, ch * CW:(ch + 1) * CW]
            )
            for n in range(NCH):
                nc.tensor.matmul(
                    d_ps[n],
                    lhsT=ssg_t[k][:, n * P:(n + 1) * P],
                    rhs=w,
                    start=(k == 0), stop=(k == HSCH - 1),
                )
        for n in range(NCH):
            ysl = y_sb[:, n, ch * CW:(ch + 1) * CW]
            nc.vector.tensor_add(ysl, ysl, d_ps[n])
            if ch == CCH - 1:
                nc.sync.dma_start(out=y_d[n * P:(n + 1) * P, :], in_=y_sb[:, n, :])

    ctx.close()


def build(dims, debug=False, enable_asserts=False):
    NT, C, E, H, HS = dims["NT"], dims["C"], dims["E"], dims["H"], dims["HS"]
    nc = bacc.Bacc(
        "TRN2", target_bir_lowering=False, debug=debug, enable_asserts=enable_asserts
    )
    aps = {
        "xT": nc.dram_tensor("xT", [C, NT], F32R, kind="ExternalInput").ap(),
        "xT32": nc.dram_tensor("xT32", [C, NT], F32, kind="ExternalInput").ap(),
        "rwT": nc.dram_tensor("rwT", [C, E], F32, kind="ExternalInput").ap(),
        "gate_up_w": nc.dram_tensor("gate_up_w", [E, C, 2 * H], F32R, kind="ExternalInput").ap(),
        "down_w": nc.dram_tensor("down_w", [E, H, C], F32R, kind="ExternalInput").ap(),
        "sgwT": nc.dram_tensor("sgwT", [C, HS], F32R, kind="ExternalInput").ap(),
        "suwT": nc.dram_tensor("suwT", [C, HS], F32R, kind="ExternalInput").ap(),
        "sdwT": nc.dram_tensor("sdwT", [HS, C], F32R, kind="ExternalInput").ap(),
        "y": nc.dram_tensor("y", [NT, C], F32, kind="ExternalOutput").ap(),
    }
    with tile.TileContext(nc) as tc:
        emit_moe(nc, tc, dims, aps)
    nc.compile()
    return nc


_NC_CACHE = {}
LAST_RESULTS = None


def kernel(**inputs):
    global LAST_RESULTS
    dims = {"NT": N_TOK // N_CORES, "C": C_FULL, "E": E_FULL, "H": H_FULL, "HS": HS_FULL}
    key = tuple(sorted(dims.items()))
    if key not in _NC_CACHE:
        _NC_CACHE[key] = build(dims)
    nc = _NC_CACHE[key]

    x = np.ascontiguousarray(np.asarray(inputs["x"], dtype=np.float32)).reshape(N_TOK, C_FULL)
    rwT = np.ascontiguousarray(np.asarray(inputs["router_w"], np.float32).T)
    guw = np.ascontiguousarray(np.asarray(inputs["gate_up_w"], np.float32))
    dw = np.ascontiguousarray(np.asarray(inputs["down_w"], np.float32))
    sgwT = np.ascontiguousarray(np.asarray(inputs["shared_gate_w"], np.float32).T)
    suwT = np.ascontiguousarray(np.asarray(inputs["shared_up_w"], np.float32).T)
    sdwT = np.ascontiguousarray(np.asarray(inputs["shared_down_w"], np.float32).T)

    NT = N_TOK // N_CORES
    in_maps = []
    for c in range(N_CORES):
        xT_c = np.ascontiguousarray(x[c * NT:(c + 1) * NT, :].T)
        in_maps.append({
            "xT": xT_c, "xT32": xT_c, "rwT": rwT, "gate_up_w": guw, "down_w": dw,
            "sgwT": sgwT, "suwT": suwT, "sdwT": sdwT,
        })

    trace = bool(os.environ.get("MOE_TRACE"))
    res = run_bass_kernel_spmd(nc, in_maps, list(range(N_CORES)), trace=trace)
    LAST_RESULTS = res
    y = np.concatenate([res.results[c]["y"] for c in range(N_CORES)], axis=0)
    return y.reshape(B, T, C_FULL).astype(np.float32)
